# revision 30
# baseline (speedup 1.0000x reference)
"""LocalitySelfAttention TRN2 kernel, v2 (software-pipelined, all-bf16).

B=4, N=2048, C=768, H=12, D=64.  8 cores: core c -> batch c//2, heads
6*(c%2) .. 6*(c%2)+6.  All-transposed dataflow as v1 (scores kept
[keys, queries]; softmax sums via a fused ones-column in the AV matmul).

Changes vs the 503us v1 (measured ~330us):
- Everything bf16 on the PE (inputs host-cast); fp32 PSUM accumulate.
- Head-major software pipeline: per head, per 1024-query half, 16
  key-block units flow ST -> exp -> AV with the tensor engine kept
  continuously busy (holds the 2.4GHz p-state) by weaving phase-1 qkv
  matmuls of later head-pairs (and the v projection, kb-synchronized
  into head 0) into the attention stream as filler.
- exp runs on the scalar(ACT) engine in [128,1024] tiles with SCALE
  folded into q at evacuation.  (gpsimd pow-exp offload was tried and
  measured ~127us/tile on HW - disabled.)
- softmax normalization: AV PSUM evacuated to SBUF immediately (frees
  the single pav accumulator in ~1.4us), then sums row -> DRAM ->
  [8,128] reshape -> exact DVE reciprocal (897ns; single-partition
  reciprocal costs 6.5us) -> DRAM broadcast -> one DVE multiply.
- output projection tail: per-m psum with a one-tile lag on the g3=2
  accumulation (heads 4-5 normalize last) + chunked ACT evacuation.
"""

import sys
import numpy as np

if "/opt/trn_rl_repo" not in sys.path:
    sys.path.insert(0, "/opt/trn_rl_repo")

B, N, C, H = 4, 2048, 768, 12
D = C // H          # 64
NH = 6              # heads per core
P = 128
CT = C // P         # 6 contraction tiles
KB = N // P         # 16 key blocks
HF = N // 2         # 1024-query halves
SCALE = float(D) ** -0.5  # 0.125

# exp-offload tuning: which kb units go to gpsimd (pow) instead of ACT
POOL_KBS_EARLY = ()               # gpsimd pow measured ~127us/tile on HW:
POOL_KBS_LATE = ()                # exp offload disabled, ACT does all exp
AV_LAG = 3                        # units of ST->AV lag at pass start

_CACHE = {}


def _build_program():
    import concourse.bass as bass
    import concourse.mybir as mybir
    import concourse.tile as tile
    from concourse import bacc
    from concourse.masks import make_identity

    f32 = mybir.dt.float32
    bf16 = mybir.dt.bfloat16
    Exp = mybir.ActivationFunctionType.Exp
    mult = mybir.AluOpType.mult
    add = mybir.AluOpType.add
    powop = mybir.AluOpType.pow

    nc = bacc.Bacc()
    xT = nc.dram_tensor("xT", [C, N], bf16, kind="ExternalInput")
    wqkv = nc.dram_tensor("wqkv", [C, 3 * NH * D], bf16, kind="ExternalInput")
    wproj = nc.dram_tensor("wproj", [NH * D, C], bf16, kind="ExternalInput")
    temp = nc.dram_tensor("temp", [P, NH], f32, kind="ExternalInput")
    outT = nc.dram_tensor("outT", [C, N], f32, kind="ExternalOutput")
    rnorm = nc.dram_tensor("rnorm", [2 * NH, HF], f32)  # internal: sum rows
    rnorm2 = nc.dram_tensor("rnorm2", [2 * NH, HF], f32)  # internal: recip rows

    def mm(out, lhsT, rhs, **kw):
        nc.tensor.matmul(out, lhsT, rhs, **kw)

    import itertools
    _uid = itertools.count()

    with tile.TileContext(nc) as tc:
        with (
            tc.tile_pool(name="const", bufs=1) as constp,
            tc.tile_pool(name="persist", bufs=1) as persist,
        ):
            # ---- persistent SBUF ------------------------------------------
            xts = persist.tile([P, CT, N], bf16, tag="xts")
            wqs = persist.tile([P, CT, 3 * NH * D], bf16, tag="wqs")
            qkT = persist.tile([P, 3, N], bf16, tag="qkT")   # q pair-groups
            # kT zero-padded per head: head h's 64 d-rows live at partition
            # offset (h%2)*64, other 64 rows stay zero.  This makes the ST
            # matmul a full [128,128]x[128,512] (rhs = both heads' q; the
            # zero rows null the other head), which measures ~231ns/matmul
            # vs ~272ns for the 64-partition config.
            kz = persist.tile([P, NH, N], bf16, tag="kz")
            vaug = persist.tile([P, NH, KB, D + 1], bf16, tag="vaug")
            attnT = persist.tile([P, NH // 2, N], bf16, tag="attnT")
            wp = persist.tile([P, NH * D // P, C], bf16, tag="wp")

            # ---- input DMAs (interleaved so PE can start early) -----------
            tbc = constp.tile([P, NH], f32, tag="tbc")
            nc.sync.dma_start(tbc[:, :], temp[:, :])
            for g3 in range(NH * D // P):
                nc.sync.dma_start(wp[:, g3, :], wproj[g3 * P : (g3 + 1) * P, :])
            # first ct tile split in pieces so the first qk chains' inputs
            # land (and the PE starts) a few us earlier
            nc.gpsimd.dma_start(xts[:, 0, 0:512], xT[0:P, 0:512])
            nc.gpsimd.dma_start(wqs[:, 0, 0:512], wqkv[0:P, 0:512])
            nc.gpsimd.dma_start(xts[:, 0, 512:N], xT[0:P, 512:N])
            nc.gpsimd.dma_start(wqs[:, 0, 512:], wqkv[0:P, 512:])
            for t in range(1, CT):
                nc.gpsimd.dma_start(xts[:, t, :], xT[t * P : (t + 1) * P, :])
                nc.gpsimd.dma_start(wqs[:, t, :], wqkv[t * P : (t + 1) * P, :])

            # ---- setup: temperature diag masks (1 - t_h * I) --------------
            ident = constp.tile([P, P], f32, tag="ident")
            make_identity(nc, ident[:])
            ntb = constp.tile([P, NH], f32, tag="ntb")
            nc.vector.tensor_scalar_mul(ntb[:, :], tbc[:, :], -1.0)
            masks = constp.tile([P, NH, P], f32, tag="masks")
            for h in range(NH):
                nc.vector.tensor_scalar(
                    masks[:, h, :], ident[:], ntb[:, h : h + 1], 1.0, mult, add
                )
            etile = constp.tile([P, HF], f32, tag="etile")
            nc.vector.memset(etile[:], float(np.e))
            onesrc = constp.tile([P, NH * KB], f32, tag="onesrc")
            nc.vector.memset(onesrc[:], 1.0)
            nc.vector.tensor_copy(
                vaug[:, :, :, D : D + 1],
                onesrc[:].rearrange("p (a b c) -> p a b c", a=NH, b=KB),
            )

            with (
                tc.tile_pool(name="pst", bufs=2, space=bass.MemorySpace.PSUM) as pstp,
                tc.tile_pool(name="pav", bufs=1, space=bass.MemorySpace.PSUM) as pavp,
                tc.tile_pool(name="ph1", bufs=2, space=bass.MemorySpace.PSUM) as ph1p,
                tc.tile_pool(name="pt", bufs=8) as ptp,
                tc.tile_pool(name="rb", bufs=2) as rbp,
                tc.tile_pool(name="srow", bufs=2) as srp,
                tc.tile_pool(name="uav", bufs=2) as uavp,
                tc.tile_pool(name="stc", bufs=2) as stcp,
            ):
                # ---- phase-1 emitters -------------------------------------
                def qk_chain(side, pg, c512):
                    """One [128,512] column chunk of a q/k pair-group.

                    side 0=q (scaled by SCALE at evac), 1=k.
                    Emits 6 matmuls (ct contraction) + 1 evacuation.
                    """
                    g = 3 * side + pg
                    col0 = side * NH * D + pg * P
                    steps = []
                    ps_box = [None]

                    def alloc():
                        ps_box[0] = ph1p.tile([P, 512], f32, tag="ph1", name=f"ph1_{next(_uid)}")

                    def emit_mm(t):
                        ps = ps_box[0]
                        mm(
                            ps[:, :],
                            wqs[:, t, col0 : col0 + P],
                            xts[:, t, c512 * 512 : (c512 + 1) * 512],
                            start=(t == 0),
                            stop=(t == CT - 1),
                        )

                    def evac():
                        ps = ps_box[0]
                        dst = qkT[:, g, c512 * 512 : (c512 + 1) * 512]
                        if side == 0:
                            nc.vector.tensor_scalar_mul(dst, ps[:], SCALE)
                        else:
                            nc.vector.tensor_copy(dst, ps[:])

                    steps.append(alloc)
                    for t in range(CT):
                        steps.append(lambda t=t: emit_mm(t))
                    steps.append(evac)
                    return steps

                def qk_group(side, pg):
                    steps = []
                    for c512 in range(4):
                        steps += qk_chain(side, pg, c512)
                    return steps

                def v_chain(rb_i):
                    """v for all 6 heads, key block rb_i: 6 mms + evac."""
                    steps = []
                    ps_box = [None]

                    def alloc():
                        ps_box[0] = ph1p.tile([P, 512], f32, tag="ph1", name=f"ph1_{next(_uid)}")

                    def emit_mm(t):
                        ps = ps_box[0]
                        mm(
                            ps[:, 0 : NH * D // 2],
                            xts[:, t, rb_i * P : (rb_i + 1) * P],
                            wqs[:, t, 2 * NH * D : 2 * NH * D + NH * D // 2],
                            start=(t == 0),
                            stop=(t == CT - 1),
                        )

                    def emit_mm2(t):
                        ps = ps_box[0]
                        mm(
                            ps[:, 256 : 256 + NH * D // 2],
                            xts[:, t, rb_i * P : (rb_i + 1) * P],
                            wqs[:, t, 2 * NH * D + NH * D // 2 : 3 * NH * D],
                            start=(t == 0),
                            stop=(t == CT - 1),
                        )

                    def evac():
                        ps = ps_box[0]
                        nc.vector.tensor_copy(
                            vaug[:, 0 : NH // 2, rb_i, 0:D],
                            ps[:, 0 : NH * D // 2].rearrange(
                                "p (h d) -> p h d", h=NH // 2
                            ),
                        )

                    def evac2():
                        ps = ps_box[0]
                        nc.vector.tensor_copy(
                            vaug[:, NH // 2 : NH, rb_i, 0:D],
                            ps[:, 256 : 256 + NH * D // 2].rearrange(
                                "p (h d) -> p h d", h=NH // 2
                            ),
                        )

                    steps.append(alloc)
                    for t in range(CT):
                        steps.append(lambda t=t: emit_mm(t))
                    for t in range(CT):
                        steps.append(lambda t=t: emit_mm2(t))
                    steps.append(evac)
                    steps.append(evac2)
                    return steps

                # ---- lead-in: qk pair 0 only ------------------------------
                for s in qk_group(0, 0) + qk_group(1, 0):
                    s()

                # v chains are woven into head 0 kb-synchronized: v_chain(kb)
                # must complete before AV(h0, kb) which trails by AV_LAG units
                vsteps = {kb: v_chain(kb) for kb in range(KB)}

                # filler queues per head (consumed during attention units)
                filler = {h: [] for h in range(NH)}
                filler[1] = qk_group(0, 1) + qk_group(1, 1)
                filler[2] = qk_group(0, 2)
                filler[3] = qk_group(1, 2)

                # ---- attention: head-major, hf inner, kb units ------------
                for h in range(NH):
                    pg = h // 2
                    off = (h % 2) * D
                    pool_kbs = set(POOL_KBS_EARLY if h < 4 else POOL_KBS_LATE)
                    fq = filler[h]
                    fpop = [0]

                    def pace(unit_idx, fq=fq, fpop=fpop):
                        # skip the first 2 units of each pass so the new
                        # pass's STs (and thus exp) aren't delayed by filler
                        p, u = unit_idx // KB, unit_idx % KB
                        eff = p * (KB - 2) + max(0, u - 1)
                        want = eff * len(fq) // (2 * (KB - 2))
                        while fpop[0] < min(want, len(fq)):
                            fq[fpop[0]]()
                            fpop[0] += 1

                    for hf in range(2):
                        sts = {}
                        pts = {}
                        pav = pavp.tile([D + 1, HF], f32, tag="pav", name=f"pav_{next(_uid)}")

                        def emit_st(kb):
                            st = pstp.tile([P, HF], f32, tag="st", name=f"st_{next(_uid)}")
                            for qc in range(2):
                                mm(
                                    st[:, qc * 512 : (qc + 1) * 512],
                                    qkT[off : off + D, 3 + pg, kb * P : (kb + 1) * P],
                                    qkT[
                                        off : off + D,
                                        pg,
                                        hf * HF + qc * 512 : hf * HF + (qc + 1) * 512,
                                    ],
                                    start=True,
                                    stop=True,
                                )
                            if kb * P // HF == hf:
                                dcol = kb * P - hf * HF
                                nc.vector.tensor_mul(
                                    st[:, dcol : dcol + P],
                                    st[:, dcol : dcol + P],
                                    masks[:, h, :],
                                )
                            return st

                        def emit_exp(kb, st):
                            pt = ptp.tile([P, HF], bf16, tag="pt", name=f"pt_{next(_uid)}")
                            if kb in pool_kbs:
                                stc = stcp.tile([P, HF], f32, tag="stc", name=f"stc_{next(_uid)}")
                                nc.vector.tensor_copy(stc[:], st[:])
                                nc.gpsimd.tensor_tensor(
                                    pt[:], etile[:], stc[:], powop
                                )
                            else:
                                nc.scalar.activation(pt[:], st[:], Exp)
                            return pt

                        def emit_av(kb, pt):
                            for qc in range(2):
                                mm(
                                    pav[:, qc * 512 : (qc + 1) * 512],
                                    vaug[:, h, kb, :],
                                    pt[:, qc * 512 : (qc + 1) * 512],
                                    start=(kb == 0),
                                    stop=(kb == KB - 1),
                                )

                        for kb in range(KB):
                            sts[kb] = emit_st(kb)
                            pts[kb] = emit_exp(kb, sts[kb])
                            if h == 0 and hf == 0:
                                for s in vsteps[kb]:
                                    s()
                            if kb >= AV_LAG:
                                emit_av(kb - AV_LAG, pts.pop(kb - AV_LAG))
                            pace(hf * KB + kb)
                        for kb in range(KB - AV_LAG, KB):
                            emit_av(kb, pts.pop(kb))

                        # ---- normalize: rows 0..63 * recip(row 64) --------
                        # evacuate PSUM promptly (frees pav for the next
                        # pass), then normalize from SBUF off-critical-path
                        uav = uavp.tile([D + 1, HF], f32, tag="uav", name=f"uav_{next(_uid)}")
                        nc.vector.tensor_copy(uav[:, :], pav[:, :])
                        ridx = 2 * h + hf
                        nc.sync.dma_start(rnorm[ridx, :], uav[D : D + 1, :])
                        rp = srp.tile([8, P], f32, tag="rp", name=f"rp_{next(_uid)}")
                        nc.sync.dma_start(
                            rp[:, :], rnorm[ridx, :].rearrange("(a b) -> a b", a=8)
                        )
                        nc.vector.reciprocal(rp[:, :], rp[:, :])
                        nc.sync.dma_start(rnorm2[ridx, :], rp[:, :])
                        rbt = rbp.tile([D, HF], f32, tag="rb", name=f"rb_{next(_uid)}")
                        nc.sync.dma_start(
                            rbt[:, :],
                            rnorm2[ridx : ridx + 1, :].broadcast_to([D, HF]),
                        )
                        nc.vector.tensor_mul(
                            attnT[off : off + D, pg, hf * HF : (hf + 1) * HF],
                            uav[0:D, :],
                            rbt[:, :],
                        )
                    # drain any remaining filler for this head
                    while fpop[0] < len(fq):
                        fq[fpop[0]]()
                        fpop[0] += 1

            # ---- phase 3: output projection (transposed) ------------------
            # po[m]'s g3=0,1 matmuls only need heads 0-3 (ready early); the
            # g3=2 rows (heads 4-5) wait on the last normalize.  Emit with a
            # one-tile lag on g3=2 so the next tile's g3=0,1 matmuls fill the
            # PE queue while the wait resolves.
            with (
                tc.tile_pool(name="psum3", bufs=2, space=bass.MemorySpace.PSUM) as psum3,
                tc.tile_pool(name="ot", bufs=2) as otp,
            ):
                pos = {}

                def po_g01(m):
                    po = psum3.tile([P, N], f32, tag="po", name=f"po_{next(_uid)}")
                    for g3 in range(2):
                        for qc in range(4):
                            mm(
                                po[:, qc * 512 : (qc + 1) * 512],
                                wp[:, g3, m * P : (m + 1) * P],
                                attnT[:, g3, qc * 512 : (qc + 1) * 512],
                                start=(g3 == 0),
                                stop=False,
                            )
                    return po

                def po_fin(m, po):
                    for qc in range(4):
                        mm(
                            po[:, qc * 512 : (qc + 1) * 512],
                            wp[:, 2, m * P : (m + 1) * P],
                            attnT[:, 2, qc * 512 : (qc + 1) * 512],
                            start=False,
                            stop=True,
                        )
                    ot = otp.tile([P, N], f32, tag="ot", name=f"ot_{next(_uid)}")
                    for half in range(2):
                        sl = slice(half * HF, (half + 1) * HF)
                        # alternate evac engines: ACT and DVE are both idle
                        # at the tail; serializing on one of them was the
                        # phase-3 bottleneck
                        if half == 0:
                            nc.scalar.copy(ot[:, sl], po[:, sl])
                        else:
                            nc.vector.tensor_copy(ot[:, sl], po[:, sl])
                        nc.sync.dma_start(outT[m * P : (m + 1) * P, sl], ot[:, sl])

                for m in range(CT):
                    pos[m] = po_g01(m)
                    if m >= 1:
                        po_fin(m - 1, pos.pop(m - 1))
                po_fin(CT - 1, pos.pop(CT - 1))

    if not nc.is_finalized():
        nc.finalize()
    return nc


def _get_program():
    if "nc" not in _CACHE:
        _CACHE["nc"] = _build_program()
    return _CACHE["nc"]


def _in_maps(x, w_qkv, w_proj, temperature):
    from ml_dtypes import bfloat16

    t = np.asarray(temperature, dtype=np.float32).reshape(H)
    maps = []
    xTs = {}
    for c in range(8):
        b, h0 = c // 2, NH * (c % 2)
        if b not in xTs:
            xTs[b] = np.ascontiguousarray(
                np.asarray(x[b], dtype=np.float32).T
            ).astype(bfloat16)
        cols = slice(D * h0, D * h0 + NH * D)
        wq = np.concatenate(
            [w_qkv[:, cols], w_qkv[:, C:][:, cols], w_qkv[:, 2 * C :][:, cols]],
            axis=1,
        )
        maps.append(
            {
                "xT": xTs[b],
                "wqkv": np.ascontiguousarray(wq, dtype=np.float32).astype(bfloat16),
                "wproj": np.ascontiguousarray(
                    w_proj[D * h0 : D * h0 + NH * D, :], dtype=np.float32
                ).astype(bfloat16),
                "temp": np.ascontiguousarray(
                    np.broadcast_to(t[h0 : h0 + NH].reshape(1, NH), (P, NH))
                ).astype(np.float32),
            }
        )
    return maps


def _install_profile_hook():
    """The agent image's antenv lacks axon_hooks; synthesize it and register
    the ctypes NTFF hook so run_bass_kernel_spmd(trace=True) can profile."""
    import types, importlib

    if "antenv.axon_hooks" not in sys.modules:
        import antenv

        mod = types.ModuleType("antenv.axon_hooks")
        _state = {"hook": None}
        mod.set_axon_ntff_profile_hook = lambda h: _state.__setitem__("hook", h)
        mod.get_axon_ntff_profile_hook = lambda: _state["hook"]
        sys.modules["antenv.axon_hooks"] = mod
        antenv.axon_hooks = mod
    from antenv.axon_hooks import (
        get_axon_ntff_profile_hook,
        set_axon_ntff_profile_hook,
    )

    if get_axon_ntff_profile_hook() is None:
        tb = importlib.import_module("trn_agent_boot.trn_boot")
        hook = tb._ntff_profile_via_ctypes("/opt/axon/libaxon_pjrt.so")
        set_axon_ntff_profile_hook(hook)


def kernel(x, w_qkv, w_proj, b_proj, temperature, _trace=False):
    from concourse.bass_utils import run_bass_kernel_spmd

    if _trace:
        try:
            _install_profile_hook()
        except Exception as e:  # profiling is best-effort
            print(f"profile hook install failed: {e}")

    nc = _get_program()
    maps = _in_maps(
        np.asarray(x, np.float32),
        np.asarray(w_qkv, np.float32),
        np.asarray(w_proj, np.float32),
        np.asarray(temperature, np.float32),
    )
    res = run_bass_kernel_spmd(nc, maps, list(range(8)), trace=_trace)
    parts = [r["outT"] for r in res.results]
    bp = np.asarray(b_proj, np.float32)
    out = np.stack(
        [(parts[2 * b] + parts[2 * b + 1]).T + bp for b in range(B)]
    ).astype(np.float32)
    if _trace:
        _CACHE["last_result"] = res
    return out


# revision 33
# speedup vs baseline: 1.0819x; 1.0819x over previous
"""LocalitySelfAttention TRN2 kernel, v2 (software-pipelined, all-bf16).

B=4, N=2048, C=768, H=12, D=64.  8 cores: core c -> batch c//2, heads
6*(c%2) .. 6*(c%2)+6.  All-transposed dataflow as v1 (scores kept
[keys, queries]; softmax sums via a fused ones-column in the AV matmul).

Changes vs the 503us v1 (measured ~330us):
- Everything bf16 on the PE (inputs host-cast); fp32 PSUM accumulate.
- Head-major software pipeline: per head, per 1024-query half, 16
  key-block units flow ST -> exp -> AV with the tensor engine kept
  continuously busy (holds the 2.4GHz p-state) by weaving phase-1 qkv
  matmuls of later head-pairs (and the v projection, kb-synchronized
  into head 0) into the attention stream as filler.
- exp runs on the scalar(ACT) engine in [128,1024] tiles with SCALE
  folded into q at evacuation.  (gpsimd pow-exp offload was tried and
  measured ~127us/tile on HW - disabled.)
- softmax normalization: AV PSUM evacuated to SBUF immediately (frees
  the single pav accumulator in ~1.4us), then sums row -> DRAM ->
  [8,128] reshape -> exact DVE reciprocal (897ns; single-partition
  reciprocal costs 6.5us) -> DRAM broadcast -> one DVE multiply.
- output projection tail: per-m psum with a one-tile lag on the g3=2
  accumulation (heads 4-5 normalize last) + chunked ACT evacuation.
"""

import sys
import numpy as np

if "/opt/trn_rl_repo" not in sys.path:
    sys.path.insert(0, "/opt/trn_rl_repo")

B, N, C, H = 4, 2048, 768, 12
D = C // H          # 64
NH = 6              # heads per core
P = 128
CT = C // P         # 6 contraction tiles
KB = N // P         # 16 key blocks
HF = N // 2         # 1024-query halves
SCALE = float(D) ** -0.5  # 0.125

# exp-offload tuning: which kb units go to gpsimd (pow) instead of ACT
POOL_KBS_EARLY = ()               # gpsimd pow measured ~127us/tile on HW:
POOL_KBS_LATE = ()                # exp offload disabled, ACT does all exp
AV_LAG = 3                        # units of ST->AV lag at pass start

_CACHE = {}


def _build_program():
    import concourse.bass as bass
    import concourse.mybir as mybir
    import concourse.tile as tile
    from concourse import bacc
    from concourse.masks import make_identity

    f32 = mybir.dt.float32
    bf16 = mybir.dt.bfloat16
    Exp = mybir.ActivationFunctionType.Exp
    mult = mybir.AluOpType.mult
    add = mybir.AluOpType.add
    powop = mybir.AluOpType.pow

    nc = bacc.Bacc()
    xT = nc.dram_tensor("xT", [C, N], bf16, kind="ExternalInput")
    wqkv = nc.dram_tensor("wqkv", [C, 3 * NH * D], bf16, kind="ExternalInput")
    wproj = nc.dram_tensor("wproj", [NH * D, C], bf16, kind="ExternalInput")
    temp = nc.dram_tensor("temp", [P, NH], f32, kind="ExternalInput")
    outT = nc.dram_tensor("outT", [C, N], f32, kind="ExternalOutput")
    rnorm = nc.dram_tensor("rnorm", [2 * NH, HF], f32)  # internal: sum rows
    rnorm2 = nc.dram_tensor("rnorm2", [2 * NH, HF], f32)  # internal: recip rows

    def mm(out, lhsT, rhs, **kw):
        nc.tensor.matmul(out, lhsT, rhs, **kw)

    import itertools
    _uid = itertools.count()

    with tile.TileContext(nc) as tc:
        with (
            tc.tile_pool(name="const", bufs=1) as constp,
            tc.tile_pool(name="persist", bufs=1) as persist,
        ):
            # ---- persistent SBUF ------------------------------------------
            xts = persist.tile([P, CT, N], bf16, tag="xts")
            wqs = persist.tile([P, CT, 3 * NH * D], bf16, tag="wqs")
            qkT = persist.tile([P, 3, N], bf16, tag="qkT")   # q pair-groups
            # kT zero-padded per head: head h's 64 d-rows live at partition
            # offset (h%2)*64, other 64 rows stay zero.  This makes the ST
            # matmul a full [128,128]x[128,512] (rhs = both heads' q; the
            # zero rows null the other head), which measures ~231ns/matmul
            # vs ~272ns for the 64-partition config.
            kz = persist.tile([P, NH, N], bf16, tag="kz")
            vaug = persist.tile([P, NH, KB, D + 1], bf16, tag="vaug")
            attnT = persist.tile([P, NH // 2, N], bf16, tag="attnT")
            wp = persist.tile([P, NH * D // P, C], bf16, tag="wp")

            # ---- input DMAs (interleaved so PE can start early) -----------
            tbc = constp.tile([P, NH], f32, tag="tbc")
            nc.sync.dma_start(tbc[:, :], temp[:, :])
            for g3 in range(NH * D // P):
                nc.sync.dma_start(wp[:, g3, :], wproj[g3 * P : (g3 + 1) * P, :])
            # first ct tile split in pieces so the first qk chains' inputs
            # land (and the PE starts) a few us earlier
            nc.gpsimd.dma_start(xts[:, 0, 0:512], xT[0:P, 0:512])
            nc.gpsimd.dma_start(wqs[:, 0, 0:512], wqkv[0:P, 0:512])
            nc.gpsimd.dma_start(xts[:, 0, 512:N], xT[0:P, 512:N])
            nc.gpsimd.dma_start(wqs[:, 0, 512:], wqkv[0:P, 512:])
            for t in range(1, CT):
                nc.gpsimd.dma_start(xts[:, t, :], xT[t * P : (t + 1) * P, :])
                nc.gpsimd.dma_start(wqs[:, t, :], wqkv[t * P : (t + 1) * P, :])

            # zero-init kz first (DVE idle during input DMAs); k evacuations
            # only ever write each head's own 64 rows
            nc.vector.memset(kz[:], 0.0)

            # ---- setup: temperature diag masks (1 - t_h * I) --------------
            ident = constp.tile([P, P], f32, tag="ident")
            make_identity(nc, ident[:])
            ntb = constp.tile([P, NH], f32, tag="ntb")
            nc.vector.tensor_scalar_mul(ntb[:, :], tbc[:, :], -1.0)
            masks = constp.tile([P, NH, P], f32, tag="masks")
            for h in range(NH):
                nc.vector.tensor_scalar(
                    masks[:, h, :], ident[:], ntb[:, h : h + 1], 1.0, mult, add
                )
            etile = constp.tile([P, HF], f32, tag="etile")
            nc.vector.memset(etile[:], float(np.e))
            onesrc = constp.tile([P, NH * KB], f32, tag="onesrc")
            nc.vector.memset(onesrc[:], 1.0)
            nc.vector.tensor_copy(
                vaug[:, :, :, D : D + 1],
                onesrc[:].rearrange("p (a b c) -> p a b c", a=NH, b=KB),
            )

            with (
                tc.tile_pool(name="pst", bufs=2, space=bass.MemorySpace.PSUM) as pstp,
                tc.tile_pool(name="pav", bufs=1, space=bass.MemorySpace.PSUM) as pavp,
                tc.tile_pool(name="ph1", bufs=2, space=bass.MemorySpace.PSUM) as ph1p,
                tc.tile_pool(name="pt", bufs=8) as ptp,
                tc.tile_pool(name="rb", bufs=2) as rbp,
                tc.tile_pool(name="srow", bufs=2) as srp,
                tc.tile_pool(name="uav", bufs=2) as uavp,
                tc.tile_pool(name="stc", bufs=2) as stcp,
            ):
                # ---- phase-1 emitters -------------------------------------
                def qk_chain(side, pg, c512):
                    """One [128,512] column chunk of a q/k pair-group.

                    side 0=q (scaled by SCALE at evac), 1=k.
                    Emits 6 matmuls (ct contraction) + 1 evacuation.
                    """
                    g = 3 * side + pg
                    col0 = side * NH * D + pg * P
                    steps = []
                    ps_box = [None]

                    def alloc():
                        ps_box[0] = ph1p.tile([P, 512], f32, tag="ph1", name=f"ph1_{next(_uid)}")

                    def emit_mm(t):
                        ps = ps_box[0]
                        mm(
                            ps[:, :],
                            wqs[:, t, col0 : col0 + P],
                            xts[:, t, c512 * 512 : (c512 + 1) * 512],
                            start=(t == 0),
                            stop=(t == CT - 1),
                        )

                    def evac():
                        ps = ps_box[0]
                        sl = slice(c512 * 512, (c512 + 1) * 512)
                        if side == 0:
                            nc.vector.tensor_scalar_mul(
                                qkT[:, pg, sl], ps[:], SCALE
                            )
                        else:
                            nc.vector.tensor_copy(
                                kz[0:D, 2 * pg, sl], ps[0:D, :]
                            )

                    def evac2():
                        ps = ps_box[0]
                        sl = slice(c512 * 512, (c512 + 1) * 512)
                        nc.vector.tensor_copy(
                            kz[D:P, 2 * pg + 1, sl], ps[D:P, :]
                        )

                    steps.append(alloc)
                    for t in range(CT):
                        steps.append(lambda t=t: emit_mm(t))
                    steps.append(evac)
                    if side == 1:
                        steps.append(evac2)
                    return steps

                def qk_group(side, pg):
                    steps = []
                    for c512 in range(4):
                        steps += qk_chain(side, pg, c512)
                    return steps

                def v_chain(rb_i):
                    """v for all 6 heads, key block rb_i: 6 mms + evac."""
                    steps = []
                    ps_box = [None]

                    def alloc():
                        ps_box[0] = ph1p.tile([P, 512], f32, tag="ph1", name=f"ph1_{next(_uid)}")

                    def emit_mm(t):
                        ps = ps_box[0]
                        mm(
                            ps[:, 0 : NH * D // 2],
                            xts[:, t, rb_i * P : (rb_i + 1) * P],
                            wqs[:, t, 2 * NH * D : 2 * NH * D + NH * D // 2],
                            start=(t == 0),
                            stop=(t == CT - 1),
                        )

                    def emit_mm2(t):
                        ps = ps_box[0]
                        mm(
                            ps[:, 256 : 256 + NH * D // 2],
                            xts[:, t, rb_i * P : (rb_i + 1) * P],
                            wqs[:, t, 2 * NH * D + NH * D // 2 : 3 * NH * D],
                            start=(t == 0),
                            stop=(t == CT - 1),
                        )

                    def evac():
                        ps = ps_box[0]
                        nc.vector.tensor_copy(
                            vaug[:, 0 : NH // 2, rb_i, 0:D],
                            ps[:, 0 : NH * D // 2].rearrange(
                                "p (h d) -> p h d", h=NH // 2
                            ),
                        )

                    def evac2():
                        ps = ps_box[0]
                        nc.vector.tensor_copy(
                            vaug[:, NH // 2 : NH, rb_i, 0:D],
                            ps[:, 256 : 256 + NH * D // 2].rearrange(
                                "p (h d) -> p h d", h=NH // 2
                            ),
                        )

                    steps.append(alloc)
                    for t in range(CT):
                        steps.append(lambda t=t: emit_mm(t))
                    for t in range(CT):
                        steps.append(lambda t=t: emit_mm2(t))
                    steps.append(evac)
                    steps.append(evac2)
                    return steps

                # ---- lead-in: qk pair 0 only ------------------------------
                for s in qk_group(0, 0) + qk_group(1, 0):
                    s()

                # v chains are woven into head 0 kb-synchronized: v_chain(kb)
                # must complete before AV(h0, kb) which trails by AV_LAG units
                vsteps = {kb: v_chain(kb) for kb in range(KB)}

                # filler queues per head (consumed during attention units)
                filler = {h: [] for h in range(NH)}
                filler[1] = qk_group(0, 1) + qk_group(1, 1)
                filler[2] = qk_group(0, 2)
                filler[3] = qk_group(1, 2)

                # ---- attention: head-major, hf inner, kb units ------------
                for h in range(NH):
                    pg = h // 2
                    off = (h % 2) * D
                    pool_kbs = set(POOL_KBS_EARLY if h < 4 else POOL_KBS_LATE)
                    fq = filler[h]
                    fpop = [0]

                    def pace(unit_idx, fq=fq, fpop=fpop):
                        # skip the first 2 units of each pass so the new
                        # pass's STs (and thus exp) aren't delayed by filler
                        p, u = unit_idx // KB, unit_idx % KB
                        eff = p * (KB - 2) + max(0, u - 1)
                        want = eff * len(fq) // (2 * (KB - 2))
                        while fpop[0] < min(want, len(fq)):
                            fq[fpop[0]]()
                            fpop[0] += 1

                    for hf in range(2):
                        sts = {}
                        pts = {}
                        pav = pavp.tile([D + 1, HF], f32, tag="pav", name=f"pav_{next(_uid)}")

                        def emit_st(kb):
                            st = pstp.tile([P, HF], f32, tag="st", name=f"st_{next(_uid)}")
                            for qc in range(2):
                                mm(
                                    st[:, qc * 512 : (qc + 1) * 512],
                                    kz[:, h, kb * P : (kb + 1) * P],
                                    qkT[
                                        :,
                                        pg,
                                        hf * HF + qc * 512 : hf * HF + (qc + 1) * 512,
                                    ],
                                    start=True,
                                    stop=True,
                                )
                            if kb * P // HF == hf:
                                dcol = kb * P - hf * HF
                                nc.vector.tensor_mul(
                                    st[:, dcol : dcol + P],
                                    st[:, dcol : dcol + P],
                                    masks[:, h, :],
                                )
                            return st

                        def emit_exp(kb, st):
                            pt = ptp.tile([P, HF], bf16, tag="pt", name=f"pt_{next(_uid)}")
                            if kb in pool_kbs:
                                stc = stcp.tile([P, HF], f32, tag="stc", name=f"stc_{next(_uid)}")
                                nc.vector.tensor_copy(stc[:], st[:])
                                nc.gpsimd.tensor_tensor(
                                    pt[:], etile[:], stc[:], powop
                                )
                            else:
                                nc.scalar.activation(pt[:], st[:], Exp)
                            return pt

                        def emit_av(kb, pt):
                            for qc in range(2):
                                mm(
                                    pav[:, qc * 512 : (qc + 1) * 512],
                                    vaug[:, h, kb, :],
                                    pt[:, qc * 512 : (qc + 1) * 512],
                                    start=(kb == 0),
                                    stop=(kb == KB - 1),
                                )

                        for kb in range(KB):
                            sts[kb] = emit_st(kb)
                            pts[kb] = emit_exp(kb, sts[kb])
                            if h == 0 and hf == 0:
                                for s in vsteps[kb]:
                                    s()
                            if kb >= AV_LAG:
                                emit_av(kb - AV_LAG, pts.pop(kb - AV_LAG))
                            pace(hf * KB + kb)
                        for kb in range(KB - AV_LAG, KB):
                            emit_av(kb, pts.pop(kb))

                        # ---- normalize: rows 0..63 * recip(row 64) --------
                        # evacuate PSUM promptly (frees pav for the next
                        # pass), then normalize from SBUF off-critical-path
                        uav = uavp.tile([D + 1, HF], f32, tag="uav", name=f"uav_{next(_uid)}")
                        nc.vector.tensor_copy(uav[:, :], pav[:, :])
                        ridx = 2 * h + hf
                        nc.sync.dma_start(rnorm[ridx, :], uav[D : D + 1, :])
                        rp = srp.tile([8, P], f32, tag="rp", name=f"rp_{next(_uid)}")
                        nc.sync.dma_start(
                            rp[:, :], rnorm[ridx, :].rearrange("(a b) -> a b", a=8)
                        )
                        nc.vector.reciprocal(rp[:, :], rp[:, :])
                        nc.sync.dma_start(rnorm2[ridx, :], rp[:, :])
                        rbt = rbp.tile([D, HF], f32, tag="rb", name=f"rb_{next(_uid)}")
                        nc.sync.dma_start(
                            rbt[:, :],
                            rnorm2[ridx : ridx + 1, :].broadcast_to([D, HF]),
                        )
                        nc.vector.tensor_mul(
                            attnT[off : off + D, pg, hf * HF : (hf + 1) * HF],
                            uav[0:D, :],
                            rbt[:, :],
                        )
                    # drain any remaining filler for this head
                    while fpop[0] < len(fq):
                        fq[fpop[0]]()
                        fpop[0] += 1

            # ---- phase 3: output projection (transposed) ------------------
            # po[m]'s g3=0,1 matmuls only need heads 0-3 (ready early); the
            # g3=2 rows (heads 4-5) wait on the last normalize.  Emit with a
            # one-tile lag on g3=2 so the next tile's g3=0,1 matmuls fill the
            # PE queue while the wait resolves.
            with (
                tc.tile_pool(name="psum3", bufs=2, space=bass.MemorySpace.PSUM) as psum3,
                tc.tile_pool(name="ot", bufs=2) as otp,
            ):
                pos = {}

                def po_g01(m):
                    po = psum3.tile([P, N], f32, tag="po", name=f"po_{next(_uid)}")
                    for g3 in range(2):
                        for qc in range(4):
                            mm(
                                po[:, qc * 512 : (qc + 1) * 512],
                                wp[:, g3, m * P : (m + 1) * P],
                                attnT[:, g3, qc * 512 : (qc + 1) * 512],
                                start=(g3 == 0),
                                stop=False,
                            )
                    return po

                def po_fin(m, po):
                    for qc in range(4):
                        mm(
                            po[:, qc * 512 : (qc + 1) * 512],
                            wp[:, 2, m * P : (m + 1) * P],
                            attnT[:, 2, qc * 512 : (qc + 1) * 512],
                            start=False,
                            stop=True,
                        )
                    ot = otp.tile([P, N], f32, tag="ot", name=f"ot_{next(_uid)}")
                    for half in range(2):
                        sl = slice(half * HF, (half + 1) * HF)
                        # alternate evac engines: ACT and DVE are both idle
                        # at the tail; serializing on one of them was the
                        # phase-3 bottleneck
                        if half == 0:
                            nc.scalar.copy(ot[:, sl], po[:, sl])
                        else:
                            nc.vector.tensor_copy(ot[:, sl], po[:, sl])
                        nc.sync.dma_start(outT[m * P : (m + 1) * P, sl], ot[:, sl])

                for m in range(CT):
                    pos[m] = po_g01(m)
                    if m >= 1:
                        po_fin(m - 1, pos.pop(m - 1))
                po_fin(CT - 1, pos.pop(CT - 1))

    if not nc.is_finalized():
        nc.finalize()
    return nc


def _get_program():
    if "nc" not in _CACHE:
        _CACHE["nc"] = _build_program()
    return _CACHE["nc"]


def _in_maps(x, w_qkv, w_proj, temperature):
    from ml_dtypes import bfloat16

    t = np.asarray(temperature, dtype=np.float32).reshape(H)
    maps = []
    xTs = {}
    for c in range(8):
        b, h0 = c // 2, NH * (c % 2)
        if b not in xTs:
            xTs[b] = np.ascontiguousarray(
                np.asarray(x[b], dtype=np.float32).T
            ).astype(bfloat16)
        cols = slice(D * h0, D * h0 + NH * D)
        wq = np.concatenate(
            [w_qkv[:, cols], w_qkv[:, C:][:, cols], w_qkv[:, 2 * C :][:, cols]],
            axis=1,
        )
        maps.append(
            {
                "xT": xTs[b],
                "wqkv": np.ascontiguousarray(wq, dtype=np.float32).astype(bfloat16),
                "wproj": np.ascontiguousarray(
                    w_proj[D * h0 : D * h0 + NH * D, :], dtype=np.float32
                ).astype(bfloat16),
                "temp": np.ascontiguousarray(
                    np.broadcast_to(t[h0 : h0 + NH].reshape(1, NH), (P, NH))
                ).astype(np.float32),
            }
        )
    return maps


def _install_profile_hook():
    """The agent image's antenv lacks axon_hooks; synthesize it and register
    the ctypes NTFF hook so run_bass_kernel_spmd(trace=True) can profile."""
    import types, importlib

    if "antenv.axon_hooks" not in sys.modules:
        import antenv

        mod = types.ModuleType("antenv.axon_hooks")
        _state = {"hook": None}
        mod.set_axon_ntff_profile_hook = lambda h: _state.__setitem__("hook", h)
        mod.get_axon_ntff_profile_hook = lambda: _state["hook"]
        sys.modules["antenv.axon_hooks"] = mod
        antenv.axon_hooks = mod
    from antenv.axon_hooks import (
        get_axon_ntff_profile_hook,
        set_axon_ntff_profile_hook,
    )

    if get_axon_ntff_profile_hook() is None:
        tb = importlib.import_module("trn_agent_boot.trn_boot")
        hook = tb._ntff_profile_via_ctypes("/opt/axon/libaxon_pjrt.so")
        set_axon_ntff_profile_hook(hook)


def kernel(x, w_qkv, w_proj, b_proj, temperature, _trace=False):
    from concourse.bass_utils import run_bass_kernel_spmd

    if _trace:
        try:
            _install_profile_hook()
        except Exception as e:  # profiling is best-effort
            print(f"profile hook install failed: {e}")

    nc = _get_program()
    maps = _in_maps(
        np.asarray(x, np.float32),
        np.asarray(w_qkv, np.float32),
        np.asarray(w_proj, np.float32),
        np.asarray(temperature, np.float32),
    )
    res = run_bass_kernel_spmd(nc, maps, list(range(8)), trace=_trace)
    parts = [r["outT"] for r in res.results]
    bp = np.asarray(b_proj, np.float32)
    out = np.stack(
        [(parts[2 * b] + parts[2 * b + 1]).T + bp for b in range(B)]
    ).astype(np.float32)
    if _trace:
        _CACHE["last_result"] = res
    return out
